# revision 70
# baseline (speedup 1.0000x reference)
"""Fused LayerNorm + single-head self-attention kernel for Trainium2 (8 NeuronCores).

Problem: x[4,64,64,128] -> LN(ch) -> QKV proj -> softmax(QK^T/sqrt(C)) V -> out proj.

Sharding: 2 cores per batch element. Each core computes its batch's full K/V
(4096 tokens) and one half of the queries (2048 rows). The host rotates each
core's batch so its query half leads (attention is invariant to k/v token
order), so queries are just x token-tiles 0..15 and the tokens are only
LayerNorm'd once; the SPMD program is uniform and needs no collectives.

Host folds gamma/beta and the 1/sqrt(C) softmax scale into the projection
weights, so the device LN is just (x-mu)*rstd.

The kernel is ACT-bound: the softmax-exp stream is ~67us of Activation-engine
time; everything else is scheduled around keeping that stream gapless:
  - all x DMAs issue up front: g0 split across the SP-HWDGE and Pool-SWDGE
    queues, g1 on SP, g2/g3 on SWDGE after the group-0 LN applies (SWDGE
    descriptor-gen occupies the Pool engine). HWDGE descriptor-gen is one
    shared serialized device, so the queue order there is x-g0, ident, wk,
    wq, bq, g1, then the late consts. DMA-completion semaphores take ~900ns
    to propagate, so each consumer is placed accordingly.
  - startup ramp at per-tile granularity: stats/rstd/apply/transpose/evac of
    tile 0 feed a [128,128] kproj/qproj/score so the first (small) exp fires
    ~6.5us in; subsequent exps widen to [128,384]/[128,512] as tiles 1-7
    arrive, reaching full [128,1024] width from kv tile 4.
  - ACT runs (almost) nothing but exp; LN applies on Pool, stats on DVE;
    rstd pairs are emitted ahead of their applies. PSUM evacuations are
    DVE-only mid-stream (Pool cannot access PSUM) except where the
    otherwise-idle ACT takes them during startup/drain.
  - scores run one kv-half behind kv production so the PE->DVE->PE chain
    never gates the exp stream; kp lives in a separate PSUM pool so the two
    score PSUM bufs only ever rotate scores.
  - attn@v pieces are [0,8)/[8,16)/[16,24)/[24,32) per query subtile j, one
    piece (8 matmuls) per exp window so no window overcommits PE.
  - v carries a ones column so attn@v also yields softmax denominators;
    normalization happens after the Wo projection (row scale commutes).
  - out chains transpose the f32 accumulator directly (no bf16 pre-cast):
    PE f32-transpose -> evac -> Wo matmul -> scale+bias.
  - tail: the last two kv tiles of block 1 are exp'd per q-half (h0 first),
    so the j=0..3 output chains launch ~1.2us before the stream ends; final
    [24,32) pieces and chains alternate PSUM pools and spread their
    evacuations over ACT (idle once exp ends) and DVE.
"""

import os
import sys
from contextlib import ExitStack

import numpy as np

for _p in ("/opt/trn_rl_repo", "/root/.axon_site/_ro/trn_rl_repo"):
    if os.path.isdir(_p) and _p not in sys.path:
        sys.path.insert(0, _p)

import concourse.bass as bass
import concourse.tile as tile
from concourse import bacc, mybir
from concourse.bass import ds, ts
from concourse._compat import with_exitstack
from concourse.bass_utils import run_bass_kernel_spmd

B, HH, WW, C = 4, 64, 64, 128
S = HH * WW  # 4096 tokens per batch
SQ = S // 2  # 2048 query rows per core
P = 128
NT = S // P  # 32 kv token tiles
QBLK = 1024
NBLK = SQ // QBLK  # 2 query blocks per core
NJ = QBLK // P  # 8 query subtiles per block
EPS = 1e-5

F32 = mybir.dt.float32
BF16 = mybir.dt.bfloat16


@with_exitstack
def _attention_kernel(ctx: ExitStack, tc: tile.TileContext, aps: dict):
    nc = tc.nc
    x, out = aps["x"], aps["out"]

    consts = ctx.enter_context(tc.tile_pool(name="consts", bufs=1))
    bigp = ctx.enter_context(tc.tile_pool(name="big", bufs=1))
    lnp = ctx.enter_context(tc.tile_pool(name="ln", bufs=4))
    statp = ctx.enter_context(tc.tile_pool(name="stat", bufs=3))
    nxp = ctx.enter_context(tc.tile_pool(name="nx", bufs=6))
    expp = ctx.enter_context(tc.tile_pool(name="expp", bufs=2))
    aop = ctx.enter_context(tc.tile_pool(name="aop", bufs=9))
    smallp = ctx.enter_context(tc.tile_pool(name="smallp", bufs=8))
    outp = ctx.enter_context(tc.tile_pool(name="outp", bufs=6))
    sap = ctx.enter_context(tc.tile_pool(name="sap", bufs=18))
    # PSUM plan (8 banks):
    #   u1: [128,1024] x2 bufs = 4 banks  (scores; tail out-proj alternate)
    #   u2: [128,4,128] x2 bufs = 2 banks (transposes, v proj, out proj)
    #   u3: [128,129]..[128,512] x2 bufs = 2 banks (k/q proj, attnv pieces)
    u1 = ctx.enter_context(tc.tile_pool(name="u1", bufs=2, space="PSUM"))
    u2 = ctx.enter_context(tc.tile_pool(name="u2", bufs=2, space="PSUM"))
    u3 = ctx.enter_context(tc.tile_pool(name="u3", bufs=2, space="PSUM"))

    def tile_ap(src_t, t0, n):
        return (src_t[t0 * P:(t0 + n) * P, :]
                .rearrange("(i p) c -> p i c", p=P))

    # ---- all x loads up front --------------------------------------------
    # g0 split across SP-HWDGE (tiles 0-3, on the first-exp critical path)
    # and Pool-SWDGE (tiles 4-7); g1 on SP behind the small consts; g2/g3
    # SWDGE descriptor-gens are emitted after the group-0 applies so they
    # don't occupy the Pool engine when the applies become ready.
    xgs = {}
    xg0 = lnp.tile([P, 8, C], F32, tag="xg")
    nc.sync.dma_start(out=xg0[:, 0:4, :], in_=tile_ap(x, 0, 4))
    nc.gpsimd.dma_start(out=xg0[:, 4:8, :], in_=tile_ap(x, 4, 4))
    xgs[0] = xg0

    idf = consts.tile([P, P], F32, tag="idf")
    nc.sync.dma_start(out=idf, in_=aps["ident"])
    w_b = {}
    wf_f = {}
    for name in ("wk", "wq"):
        wf = consts.tile([C, C], F32, tag=f"{name}_f")
        nc.sync.dma_start(out=wf, in_=aps[name])
        wf_f[name] = wf
    bq_s = consts.tile([C, 1], F32, tag="bq")
    nc.sync.dma_start(out=bq_s, in_=aps["bq"])
    xg1 = lnp.tile([P, 8, C], F32, tag="xg")
    nc.sync.dma_start(out=xg1[:, :, :], in_=tile_ap(x, 8, 8))
    xgs[1] = xg1
    # g2/g3 also ride the SP HWDGE queue, behind g1: SWDGE descriptor-gen
    # would occupy the Pool engine right when the LN applies become ready,
    # and their transfers would cut ahead of the weight DMAs on the
    # serialized DMA-engines device
    xg2 = lnp.tile([P, 8, C], F32, tag="xg")
    nc.sync.dma_start(out=xg2[:, :, :], in_=tile_ap(x, 16, 8))
    xgs[2] = xg2
    xg3 = lnp.tile([P, 8, C], F32, tag="xg")
    nc.sync.dma_start(out=xg3[:, :, :], in_=tile_ap(x, 24, 8))
    xgs[3] = xg3

    eps_t = consts.tile([P, 1], F32, tag="eps")
    nc.vector.memset(eps_t, EPS)

    # bf16 casts on Pool ahead of the applies (wq's cast is emitted after
    # the first applies: it is not needed until the first qproj)
    id_b = consts.tile([P, P], BF16, tag="idb")
    nc.gpsimd.tensor_copy(id_b, idf)
    wb = consts.tile([C, C], BF16, tag="wk_b")
    nc.gpsimd.tensor_copy(wb, wf_f["wk"])
    w_b["wk"] = wb

    def late_consts():
        for name in ("wv", "wo"):
            wf = consts.tile([C, C], F32, tag=f"{name}_f")
            nc.sync.dma_start(out=wf, in_=aps[name])
            wb = consts.tile([C, C], BF16, tag=f"{name}_b")
            nc.gpsimd.tensor_copy(wb, wf)
            w_b[name] = wb
        bob = consts.tile([P, C], F32, tag="bob")
        nc.sync.dma_start(out=bob, in_=aps["bob"])
        return bob

    # --- big persistent SBUF tensors
    nxT = bigp.tile([P, S], BF16, tag="nxT")      # normalized x, transposed
    kT = bigp.tile([P, S], BF16, tag="kT")
    qT = bigp.tile([P, SQ], BF16, tag="qT")
    vsb = bigp.tile([P, NT, 130], BF16, tag="vsb")  # [tok, c] + ones col at 128
    nc.vector.memset(vsb[:, :, 128:129], 1.0)
    eTs = []
    for _bi in range(NBLK):
        eT_blk = expp.tile([P, NT, QBLK], BF16, tag="eT")
        eTs.append(eT_blk)

    def emit_rstd(rstd, mv, sl):
        # rstd = exp(-0.5*ln(var+eps)); Ln and Exp share one activation
        # table set so this never reloads tables mid-stream
        nc.scalar.activation(
            rstd[:, sl], mv[:, sl, 1],
            func=mybir.ActivationFunctionType.Ln,
            bias=eps_t, scale=1.0)
        nc.scalar.activation(
            rstd[:, sl], rstd[:, sl],
            func=mybir.ActivationFunctionType.Exp,
            scale=-0.5)

    def ln_stats(g):
        # stats (DVE) + rstd (ACT) for one 8-tile group; the DMA was issued
        # up front. Emitted several score-tiles ahead of ln_apply so the
        # rstd pair sits early in the ACT queue.
        xg = xgs[g]
        st = statp.tile([P, 8, 6], F32, tag="st")
        mv = statp.tile([P, 8, 2], F32, tag="mv")
        rstd = statp.tile([P, 8], F32, tag="rstd")
        for i in range(8):
            nc.vector.bn_stats(st[:, i, :], xg[:, i, :])
            nc.vector.bn_aggr(mv[:, i, :], st[:, i, :])
        emit_rstd(rstd, mv, slice(0, 8))
        return xg, mv, rstd

    def ln_apply(state, g, dstT, half_hook=None):
        # normalize (Pool), PE-transpose into dstT columns, DVE evacuation
        xg, mv, rstd = state
        for half in range(2):
            tp = u2.tile([P, 4, P], F32, tag="u2")
            for i in range(4 * half, 4 * half + 4):
                nxt = nxp.tile([P, C], BF16, tag="nxt")
                nc.gpsimd.tensor_scalar(
                    nxt, xg[:, i, :], mv[:, i, 0:1], rstd[:, i:i + 1],
                    mybir.AluOpType.subtract, mybir.AluOpType.mult)
                nc.tensor.matmul(tp[:, i % 4, :], lhsT=nxt, rhs=id_b,
                                 start=True, stop=True)
            base = (g * 8 + 4 * half) * P
            nc.vector.tensor_copy(dstT[:, ds(base, 4 * P)], tp)
            if half_hook is not None:
                half_hook(half)

    def emit_qproj_half(j, h):
        qp = u3.tile([P, 512], F32, tag="u3")
        nc.tensor.matmul(qp, lhsT=w_b["wq"],
                         rhs=nxT[:, ds(j * QBLK + h * 512, 512)],
                         start=True, stop=True)
        nc.vector.tensor_scalar(
            qT[:, ds(j * QBLK + h * 512, 512)], qp, bq_s, None,
            mybir.AluOpType.add)

    def emit_scores(b, i):
        sp = u1.tile([P, QBLK], F32, tag="u1")
        for h in range(2):
            nc.tensor.matmul(sp[:, ts(h, 512)], lhsT=kT[:, ts(i, P)],
                             rhs=qT[:, ds(b * QBLK + h * 512, 512)],
                             start=True, stop=True)
        nc.scalar.activation(eTs[b][:, i, :], sp,
                             func=mybir.ActivationFunctionType.Exp)

    def emit_scores_half(b, i, h):
        sp = u1.tile([P, 512], F32, tag="u1")
        nc.tensor.matmul(sp, lhsT=kT[:, ts(i, P)],
                         rhs=qT[:, ds(b * QBLK + h * 512, 512)],
                         start=True, stop=True)
        nc.scalar.activation(eTs[b][:, i, ds(h * 512, 512)], sp,
                             func=mybir.ActivationFunctionType.Exp)

    def v_half(g, half):
        base = g * 8 + 4 * half
        vp = u2.tile([P, 4, C], F32, tag="u2")
        for i in range(4):
            nc.tensor.matmul(vp[:, i, :], lhsT=nxT[:, ts(base + i, P)],
                             rhs=w_b["wv"], start=True, stop=True)
        nc.vector.tensor_copy(vsb[:, ds(base, 4), 0:128], vp)

    def kv_half(g, half, with_v=True):
        # kp lives in u3, not u1: sharing the two score bufs would insert a
        # non-score consumer into the exp-paced rotation and hiccup ACT
        base = g * 8 + 4 * half
        kp = u3.tile([P, 512], F32, tag="u3")
        nc.tensor.matmul(kp, lhsT=w_b["wk"], rhs=nxT[:, ds(base * P, 512)],
                         start=True, stop=True)
        nc.vector.tensor_copy(kT[:, ds(base * P, 512)], kp)
        if with_v:
            v_half(g, half)

    # attn@v piece: accumulate kv tiles [t0,t1) for query subtile j of block
    # b into PSUM, then fold into the per-j SBUF accumulator on DVE. The
    # last piece produces `tot` and triggers the output chain.
    sA = {}
    ot_pairs = {}

    def attnv_piece(b, j, t0, t1, bob_s, defer=False, alt_psum=False,
                    inject=False):
        pool = u2 if alt_psum else u3
        opp = pool.tile([P, 129], F32, tag="u2" if alt_psum else "u3")
        for i in range(t0, t1):
            nc.tensor.matmul(opp, lhsT=eTs[b][:, i, ts(j, P)],
                             rhs=vsb[:, i, 0:129],
                             start=(i == t0), stop=(i == t1 - 1))
        if t0 == 0:
            s = sap.tile([P, 129], F32, tag="sA")
            nc.vector.tensor_copy(s, opp)
            sA[(b, j)] = s
        elif t1 < NT:
            nc.vector.tensor_add(sA[(b, j)], opp, sA[(b, j)])
        else:
            tot = aop.tile([P, 129], F32, tag="tot")
            nc.vector.tensor_add(tot, opp, sA.pop((b, j)))
            if defer:
                return tot
            out_chain(b, j, tot, bob_s)
        return None

    def out_chain(b, j, tot, bob_s, tail=False):
        # normalize AFTER the Wo projection (row scale commutes with
        # matmul): the reciprocal runs concurrently with transpose+Wo.
        # The f32 accumulator transposes directly (is_transpose, 2cyc/row);
        # no bf16 pre-cast needed.
        r = smallp.tile([P, 1], F32, tag="r")
        nc.vector.reciprocal(r, tot[:, 128:129])
        if tail and j % 2 == 1:
            # tail: score PSUM banks are free; alternating pools doubles
            # the number of in-flight output chains
            tfp = u1.tile([P, 4, C], F32, tag="u1")
        else:
            tfp = u2.tile([P, 4, C], F32, tag="u2")
        nc.tensor.transpose(tfp[:, 0, :], tot[:, 0:128], idf)
        aoT = aop.tile([P, C], BF16, tag="aoT")
        if tail:
            # the exp stream is over: ACT is free to evacuate PSUM
            nc.scalar.copy(aoT, tfp[:, 0, :])
        else:
            nc.vector.tensor_copy(aoT, tfp[:, 0, :])
        nc.tensor.matmul(tfp[:, 1, :], lhsT=aoT, rhs=w_b["wo"],
                         start=True, stop=True)
        if j % 2 == 0:
            ot_pair = outp.tile([P, 2, C], F32, tag="ot")
            ot_pairs[b] = ot_pair
        ot = ot_pairs[b]
        nc.vector.scalar_tensor_tensor(
            ot[:, j % 2, :], tfp[:, 1, :], r, bob_s,
            mybir.AluOpType.mult, mybir.AluOpType.add)
        if tail and j >= NJ - 2:
            # the very last two outputs go out individually, each on its own
            # idle generator, so the final DMA's descriptor-gen starts the
            # moment its data lands instead of waiting for the pair
            eng = nc.sync if j == NJ - 1 else nc.gpsimd
            eng.dma_start(out=out[ds(b * QBLK + j * P, P), :],
                          in_=ot[:, j % 2, :])
        elif j % 2 == 1:
            # one DMA per chain pair (rows are adjacent): halves the
            # descriptor-generation serialization at the drain
            nc.sync.dma_start(
                out=out[ds(b * QBLK + (j - 1) * P, 2 * P), :]
                .rearrange("(i p) c -> p i c", p=P),
                in_=ot)

    # ---- startup ramp ----------------------------------------------------
    # Per-tile LN of tiles 0-3 feeds a [128,128] kproj/qproj/score chain so
    # the first exp fires as early as possible, then widths grow with
    # supply. Each tile's evacuation is emitted right after its transpose so
    # its engine-counter dependency covers only that one PE instruction.
    st0 = statp.tile([P, 8, 6], F32, tag="st")
    mv0 = statp.tile([P, 8, 2], F32, tag="mv")
    rstd0 = statp.tile([P, 8], F32, tag="rstd")
    for i in range(4):
        nc.vector.bn_stats(st0[:, i, :], xg0[:, i, :])
        nc.vector.bn_aggr(mv0[:, i, :], st0[:, i, :])
    emit_rstd(rstd0, mv0, slice(0, 1))
    # floor: the scheduler's internal sim under-models DMA latency and would
    # queue this Ln ahead of the (0:1) Exp on ACT, delaying tile-0's apply
    # by ~0.9us; the floor is below the pair's real dependency-ready time so
    # it costs nothing at runtime
    with tc.tile_wait_until(0.0044):
        emit_rstd(rstd0, mv0, slice(1, 4))

    wb = consts.tile([C, C], BF16, tag="wq_b")
    nc.gpsimd.tensor_copy(wb, wf_f["wq"])
    w_b["wq"] = wb
    tp0 = u2.tile([P, 4, P], F32, tag="u2")
    for i in range(4):
        nxt = nxp.tile([P, C], BF16, tag="nxt")
        nc.gpsimd.tensor_scalar(
            nxt, xg0[:, i, :], mv0[:, i, 0:1], rstd0[:, i:i + 1],
            mybir.AluOpType.subtract, mybir.AluOpType.mult)
        nc.tensor.matmul(tp0[:, i, :], lhsT=nxt, rhs=id_b,
                         start=True, stop=True)
        if i in (0, 2):
            nc.scalar.copy(nxT[:, ts(i, P)], tp0[:, i, :])
        else:
            nc.vector.tensor_copy(nxT[:, ts(i, P)], tp0[:, i, :])
        if i == 0:
            # tile-0 k/q/score/exp chain, narrowest possible
            kp0 = u3.tile([P, 128], F32, tag="u3")
            nc.tensor.matmul(kp0, lhsT=w_b["wk"], rhs=nxT[:, 0:128],
                             start=True, stop=True)
            nc.vector.tensor_copy(kT[:, 0:128], kp0)
            qp0 = u3.tile([P, 128], F32, tag="u3")
            nc.tensor.matmul(qp0, lhsT=w_b["wq"], rhs=nxT[:, 0:128],
                             start=True, stop=True)
            nc.scalar.activation(
                qT[:, 0:128], qp0,
                func=mybir.ActivationFunctionType.Identity, bias=bq_s)
            sp = u1.tile([P, 128], F32, tag="u1")
            nc.tensor.matmul(sp, lhsT=kT[:, 0:128], rhs=qT[:, 0:128],
                             start=True, stop=True)
            nc.scalar.activation(eTs[0][:, 0, 0:128], sp,
                                 func=mybir.ActivationFunctionType.Exp)

    # widen: k tiles 1-3, then q 128:512, then (kv0,q128:512), (kv1-3,q0:512)
    kp1 = u3.tile([P, 384], F32, tag="u3")
    nc.tensor.matmul(kp1, lhsT=w_b["wk"], rhs=nxT[:, 128:512],
                     start=True, stop=True)
    nc.vector.tensor_copy(kT[:, 128:512], kp1)
    qp1 = u3.tile([P, 384], F32, tag="u3")
    nc.tensor.matmul(qp1, lhsT=w_b["wq"], rhs=nxT[:, 128:512],
                     start=True, stop=True)
    nc.vector.tensor_scalar(qT[:, 128:512], qp1, bq_s, None,
                            mybir.AluOpType.add)
    sp = u1.tile([P, 384], F32, tag="u1")
    nc.tensor.matmul(sp, lhsT=kT[:, 0:128], rhs=qT[:, 128:512],
                     start=True, stop=True)
    nc.scalar.activation(eTs[0][:, 0, 128:512], sp,
                         func=mybir.ActivationFunctionType.Exp)
    for i in range(1, 4):
        sp = u1.tile([P, 512], F32, tag="u1")
        nc.tensor.matmul(sp, lhsT=kT[:, ts(i, P)], rhs=qT[:, 0:512],
                         start=True, stop=True)
        nc.scalar.activation(eTs[0][:, i, 0:512], sp,
                             func=mybir.ActivationFunctionType.Exp)

    # group0 half 1 (tiles 4-7): stats/rstd/apply/transpose/evac, then
    # k/q h1 and the q512:1024 scores for kv tiles 0-3
    for i in range(4, 8):
        nc.vector.bn_stats(st0[:, i, :], xg0[:, i, :])
        nc.vector.bn_aggr(mv0[:, i, :], st0[:, i, :])
    with tc.tile_wait_until(0.0056):
        emit_rstd(rstd0, mv0, slice(4, 8))
    tp1 = u2.tile([P, 4, P], F32, tag="u2")
    for i in range(4, 8):
        nxt = nxp.tile([P, C], BF16, tag="nxt")
        nc.gpsimd.tensor_scalar(
            nxt, xg0[:, i, :], mv0[:, i, 0:1], rstd0[:, i:i + 1],
            mybir.AluOpType.subtract, mybir.AluOpType.mult)
        nc.tensor.matmul(tp1[:, i % 4, :], lhsT=nxt, rhs=id_b,
                         start=True, stop=True)
        # all h1 evacuations on DVE: ACT is already running the ramp exps
        nc.vector.tensor_copy(nxT[:, ts(i, P)], tp1[:, i % 4, :])
    kph = u3.tile([P, 512], F32, tag="u3")
    nc.tensor.matmul(kph, lhsT=w_b["wk"], rhs=nxT[:, 512:1024],
                     start=True, stop=True)
    nc.vector.tensor_copy(kT[:, 512:1024], kph)
    qph = u3.tile([P, 512], F32, tag="u3")
    nc.tensor.matmul(qph, lhsT=w_b["wq"], rhs=nxT[:, 512:1024],
                     start=True, stop=True)
    nc.vector.tensor_scalar(qT[:, 512:1024], qph, bq_s, None,
                            mybir.AluOpType.add)
    for i in range(4):
        sp = u1.tile([P, 512], F32, tag="u1")
        nc.tensor.matmul(sp, lhsT=kT[:, ts(i, P)], rhs=qT[:, 512:1024],
                         start=True, stop=True)
        nc.scalar.activation(eTs[0][:, i, 512:1024], sp,
                             func=mybir.ActivationFunctionType.Exp)

    bob_s = late_consts()
    # group-1 LN is not needed until kv tile 8's score (~16us); a schedule
    # floor keeps its stats/applies out of the ramp's critical DVE/PE slots
    # (the list scheduler's internal sim under-models DMA-completion
    # latency and would otherwise queue them ahead of the h1 score matmuls)
    with tc.tile_wait_until(0.0115):
        states = {1: ln_stats(1)}

    # ---- steady stream ---------------------------------------------------
    # Window t emits (scores+exp for block-0 kv tile t) plus post-work chosen
    # so (a) each group's stats+rstd precede its applies by several exp
    # slots, (b) at most one attnv piece lands per exp window, (c) a piece's
    # exps are complete when PE reaches it (PE runs ~2 slots ahead of ACT).
    def post0(t):
        if t == 5:
            with tc.tile_wait_until(0.0135):
                states[2] = ln_stats(2)
        elif t == 9:
            with tc.tile_wait_until(0.0215):
                states[3] = ln_stats(3)
        elif t == 14:
            emit_qproj_half(1, 0)
        elif t == 16:
            emit_qproj_half(1, 1)
        if 10 <= t <= 17:
            attnv_piece(0, t - 10, 0, 8, bob_s)       # needs exp(0,7)
        elif 18 <= t <= 25:
            attnv_piece(0, t - 18, 8, 16, bob_s)      # needs exp(0,15)
        elif 26 <= t <= 31:
            attnv_piece(0, t - 26, 16, 24, bob_s)     # needs exp(0,23)

    def hook(g, half):
        kv_half(g, half, with_v=False)
        # v halves land ahead of this hook's scores so attnv pieces
        # scheduled in later windows never read unwritten vsb tiles
        if g == 1 and half == 0:
            v_half(0, 0)
            v_half(0, 1)
        elif g == 1:
            v_half(1, 0)
            v_half(1, 1)
        else:
            v_half(g, half)
        # scores run one half behind kv production
        prev = g * 8 + 4 * half - 4
        for i in range(prev, prev + 4):
            emit_scores(0, i)
            post0(i)

    with tc.tile_wait_until(0.0135):
        ln_apply(states.pop(1), 1, nxT,
                 half_hook=lambda half: hook(1, half))
    for g in range(2, 4):
        ln_apply(states.pop(g), g, nxT,
                 half_hook=lambda half, g=g: hook(g, half))
    for i in range(28, 32):
        emit_scores(0, i)
        post0(i)

    def post1(i):
        if i in (0, 1):
            attnv_piece(0, 6 + i, 16, 24, bob_s)      # block-0 stragglers
        if 2 <= i <= 9:
            attnv_piece(0, i - 2, 24, 32, bob_s)      # block-0 outputs
        elif 10 <= i <= 17:
            attnv_piece(1, i - 10, 0, 8, bob_s)       # needs exp(1,7)
        elif 18 <= i <= 25:
            attnv_piece(1, i - 18, 8, 16, bob_s)      # needs exp(1,15)
        elif i >= 26:
            attnv_piece(1, i - 26, 16, 24, bob_s)     # needs exp(1,23)

    for i in range(30):
        emit_scores(1, i)
        post1(i)
    # tail: the last two kv tiles are exp'd per q-half, h0 first, so the
    # j=0..3 chains launch while ACT still runs the h1 exps
    emit_scores_half(1, 30, 0)
    attnv_piece(1, 4, 16, 24, bob_s)
    emit_scores_half(1, 31, 0)
    attnv_piece(1, 5, 16, 24, bob_s)
    tots = {}
    for j in range(2):
        tots[j] = attnv_piece(1, j, 24, 32, bob_s, defer=True,
                              alt_psum=(j % 2 == 1))
    emit_scores_half(1, 30, 1)
    attnv_piece(1, 6, 16, 24, bob_s)
    tots[2] = attnv_piece(1, 2, 24, 32, bob_s, defer=True)
    out_chain(1, 0, tots.pop(0), bob_s, tail=True)
    emit_scores_half(1, 31, 1)
    attnv_piece(1, 7, 16, 24, bob_s)
    tots[3] = attnv_piece(1, 3, 24, 32, bob_s, defer=True, alt_psum=True)
    # drain: remaining final pieces, psum slots alternating u3/u2 so the
    # DVE folds pipeline; chains interleave 2 ahead. Odd-j chains borrow
    # the score PSUM pool (u1) — they are only emitted after the last
    # score-half above, so the u1 rotation never blocks an exp.
    for j in range(1, NJ):
        if j + 3 < NJ:
            tots[j + 3] = attnv_piece(1, j + 3, 24, 32, bob_s, defer=True,
                                      alt_psum=(j % 2 == 0))
        out_chain(1, j, tots.pop(j), bob_s, tail=True)


_CACHE = {}


def _patch_act_tables():
    # Force every activation onto the natural_log_exp_and_others set (it has
    # both Ln and Exp; Copy/Identity are in every set). The default chooser
    # puts Ln and Exp in different sets, and LN interleaved with the softmax
    # exp stream then reloads tables (~2.7us) on every switch. Emptying the
    # other sets preserves dict order, so act_func_set_id indices stay
    # aligned with act_info.json.
    if getattr(bacc, "_act_tables_patched", False):
        return
    orig = bacc.get_activation_tables

    def patched(module_arch):
        tabs = orig(module_arch)
        keep = "natural_log_exp_and_others"
        if keep in tabs:
            tabs = {k: (v if k == keep else type(v)()) for k, v in tabs.items()}
        return tabs

    bacc.get_activation_tables = patched
    bacc._act_tables_patched = True


def _build():
    if "nc" in _CACHE:
        return _CACHE["nc"]
    _patch_act_tables()
    nc = bacc.Bacc("TRN2", target_bir_lowering=False, debug=False, num_devices=8)
    aps = {}
    for name, shape in (
        ("x", [S, C]),
        ("wq", [C, C]), ("wk", [C, C]), ("wv", [C, C]), ("wo", [C, C]),
        ("bq", [C, 1]), ("bob", [P, C]), ("ident", [P, P]),
    ):
        aps[name] = nc.dram_tensor(name, shape, F32, kind="ExternalInput").ap()
    aps["out"] = nc.dram_tensor("out", [SQ, C], F32, kind="ExternalOutput").ap()
    with tile.TileContext(nc) as tc:
        _attention_kernel(tc, aps)
    nc.compile()
    _CACHE["nc"] = nc
    return nc


def _host_fold(gamma, beta, Wq, bq, Wk, bk, Wv, bv, Wo, bo):
    scale = 1.0 / np.sqrt(np.float32(C))
    f = {}
    f["wq"] = (gamma[:, None] * Wq * scale).astype(np.float32)
    f["bq"] = ((beta @ Wq + bq) * scale).astype(np.float32).reshape(C, 1)
    f["wk"] = (gamma[:, None] * Wk).astype(np.float32)
    f["wv"] = (gamma[:, None] * Wv).astype(np.float32)
    # v bias (incl. beta@Wv) passes through softmax untouched; fold via Wo.
    bvf = (beta @ Wv + bv).astype(np.float32)
    f["wo"] = np.asarray(Wo, dtype=np.float32)
    bof = (np.asarray(bo, np.float32) + bvf @ np.asarray(Wo, np.float32))
    f["bob"] = np.ascontiguousarray(np.broadcast_to(bof, (P, C)))
    f["ident"] = np.eye(P, dtype=np.float32)
    return f


def make_in_maps(x, gamma, beta, Wq, bq, Wk, bk, Wv, bv, Wo, bo):
    x = np.asarray(x, dtype=np.float32)
    folded = _host_fold(
        np.asarray(gamma, np.float32), np.asarray(beta, np.float32),
        np.asarray(Wq, np.float32), np.asarray(bq, np.float32),
        np.asarray(Wk, np.float32), np.asarray(bk, np.float32),
        np.asarray(Wv, np.float32), np.asarray(bv, np.float32),
        np.asarray(Wo, np.float32), np.asarray(bo, np.float32))
    xs = x.reshape(B, S, C)
    in_maps = []
    for core in range(8):
        bi, half = core // 2, core % 2
        m = dict(folded)
        # rotate so this core's query half leads; attention is invariant
        # to k/v token order, and outputs only cover the query half
        m["x"] = np.ascontiguousarray(
            np.roll(xs[bi], -half * SQ, axis=0))
        in_maps.append(m)
    return in_maps


def assemble(results):
    full = np.empty((B, S, C), dtype=np.float32)
    for core in range(8):
        bi, half = core // 2, core % 2
        full[bi, half * SQ:(half + 1) * SQ] = results[core]["out"]
    return full.reshape(B, HH, WW, C)


def kernel(x, gamma, beta, Wq, bq, Wk, bk, Wv, bv, Wo, bo):
    nc = _build()
    in_maps = make_in_maps(x, gamma, beta, Wq, bq, Wk, bk, Wv, bv, Wo, bo)
    res = run_bass_kernel_spmd(nc, in_maps, list(range(8)))
    return assemble(res.results)
